# revision 18
# baseline (speedup 1.0000x reference)
"""LFMA adapter kernel for 8 Trainium2 NeuronCores.

y = x @ W_base.T + b + alpha * x @ Re(ifft2(scatter(c)))      x:[2,64,4096]

With E[d,k] = e^{+i th d k} (th = 2pi/D) and F the dense complex scatter of c:

    y2[t,n] = (alpha/D^2) * Re( sum_{k,l} U[t,k] F[k,l] e^{i th n l} ),
    U = x @ E.

x real => U Hermitian in k: fold k onto 0..2048 via P = F[k]+F[-k],
M = F[k]-F[-k] (rows 0,2048 halved):
    v[t,l] = ur @ P[:,l] + i * ui @ M[:,l]   (l = 0..D-1)
The final Re() makes only Re(v[l]+v[-l]) and Im(v[l]-v[-l]) matter, so l
also folds onto 0..2048 (host-folded matrices, cols 0,2048 halved):
    vp = ur @ P+r - ui @ M+i          (per core: its 272-col l-share)
    vm = ur @ P-i + ui @ M-r
    y2[t,n] = gamma * sum_l [ vp[t,l] cos(th n l) - vm[t,l] sin(th n l) ]

All Fourier-path operands are fp8e4m3 (y2 is ~2e-4 of output scale, so its
error budget is enormous) and run in DoubleRow perf mode (2 contraction
tiles per instruction); the base matmul x@W^T stays bf16 (~2e-3 rel).
Stages 1 and 2 compute their outputs directly transposed ([k,t] / [l,t]
PSUM tiles via stationary A/PM operands), so the all-gather staging needs
only a PSUM->fp8 copy: no separate cast+PE-transpose round trip.
Scaling: A carries 0.5*cos/sin (u rms ~23, fp8e4m3 max 240), PM carries
16x the folded c, B carries raw cos/sin; gamma = alpha/D^2 lands in the
final PSUM combine on DVE.

Sharding (8 cores): stage 1 u k-columns (272/core, all-gather uT fp8),
stage 2 folded-v l-columns (272/core, all-gather vT fp8), stage 3 output
columns (512/core). Both all-gathers carry [2176,2,128] fp8 = 0.56MB.
DMA queues: SP carries x/W/bounceA/stores, Act carries A/PM/staging,
Pool carries B/bounceB and the two collectives; the base matmul halves
fill the PE during the all-gathers.
"""

import numpy as np
import ml_dtypes

import concourse.mybir as mybir
import concourse.tile as tile
from concourse import bacc
from concourse.bass import ts
from concourse.bass_utils import run_bass_kernel_spmd

BF16 = mybir.dt.bfloat16
F32 = mybir.dt.float32
F8 = mybir.dt.float8e4
NP_BF16 = ml_dtypes.bfloat16
NP_F8 = mybir.dt.np(mybir.dt.float8e4)
DR = mybir.MatmulPerfMode.DoubleRow

D = 4096          # d1 == d2
T = 128           # 2*64 flattened tokens
NCORES = 8
SH = D // NCORES  # 512 output columns per core
NT = D // 128     # 32 contraction tiles over d
KH = 2176         # half-spectrum range 0..2048 padded to 17*128
KT = KH // 128    # 17 k/l tiles
SHK = KH // NCORES  # 272 u-columns (and folded-v columns) per core
ALPHA = 16.0

A_SCALE = 0.5     # A holds 0.5*cos/sin -> u psum rms ~23 (fp8e4m3 max 240)
PM_SCALE = 16.0   # PM holds 16x folded c -> better fp8 relative precision
V_CAST = 1.0 / (A_SCALE * PM_SCALE)   # v psum -> v_true before fp8 cast
GAMMA = ALPHA / (D * D)               # folded into the final PSUM combine

_CACHE = {}

# stage-2/3 contraction tile groups: DoubleRow pairs within each bounce
# piece (tiles 0..5 / 6..11 / 12..16), tile 16 single.
GROUPS = [((0, 1), 0, 0),
          ((2, 3), 1, 0), ((4, 5), 1, 2), ((6, 7), 1, 4),
          ((8, 9), 2, 0), ((10, 11), 2, 2), ((12, 13), 2, 4),
          ((14, 15), 2, 6), ((16,), 2, 8)]
PIECES = [(0, 2), (2, 6), (8, 9)]   # (first tile, n tiles) per bounce piece
# transposed-output row tiles of the 272-column share
TAUS = [(0, 128), (128, 128), (256, 16)]


def _tilemaj(m, dt):
    """[128*nt, n] -> tile-major [128, nt*n] (tile i at cols i*n:(i+1)*n)."""
    rows, n = m.shape
    nt = rows // 128
    return np.ascontiguousarray(
        m.reshape(nt, 128, n).transpose(1, 0, 2).reshape(128, nt * n)
    ).astype(dt)


def _build_program(reps=1):
    nc = bacc.Bacc("TRN2", target_bir_lowering=False, debug=False,
                   num_devices=NCORES)
    AH = [128, (NT // 2) * SHK]   # stage-1 A shard halves, fp8
    PMS = [128, KT * SHK]         # stage-2 folded shard, fp8
    BS = [128, KT * SH]           # stage-3 B shard, fp8
    WS = [128, NT * SH]           # base weight shard, bf16

    xt = nc.dram_tensor("xt", [128, NT * 128], BF16, kind="ExternalInput")
    xt8 = nc.dram_tensor("xt8", [128, NT * 128], F8, kind="ExternalInput")
    a_r = nc.dram_tensor("a_r", [128, NT * SHK], F8, kind="ExternalInput")
    a_i = nc.dram_tensor("a_i", [128, NT * SHK], F8, kind="ExternalInput")
    ppr = nc.dram_tensor("ppr", PMS, F8, kind="ExternalInput")
    mpin = nc.dram_tensor("mpin", PMS, F8, kind="ExternalInput")
    pmi = nc.dram_tensor("pmi", PMS, F8, kind="ExternalInput")
    mmr = nc.dram_tensor("mmr", PMS, F8, kind="ExternalInput")
    b_c = nc.dram_tensor("b_c", BS, F8, kind="ExternalInput")
    b_sn = nc.dram_tensor("b_sn", BS, F8, kind="ExternalInput")
    w_t = nc.dram_tensor("w_t", WS, BF16, kind="ExternalInput")
    bias = nc.dram_tensor("bias", [1, SH], BF16, kind="ExternalInput")
    y_out = nc.dram_tensor("y", [T, SH], BF16, kind="ExternalOutput")

    RG = [list(range(NCORES))]
    NTH = NT // 2   # base-matmul halves (16 d-tiles each)
    AHW = (NT // 2) * SHK

    with tile.TileContext(nc) as tc:
        with (
            tc.tile_pool(name="dram", bufs=2, space="DRAM") as dramp,
            tc.tile_pool(name="const", bufs=1) as constp,
            tc.tile_pool(name="apool", bufs=2) as apool,
            tc.tile_pool(name="pmpool", bufs=2) as pmpool,
            tc.tile_pool(name="bpool", bufs=2) as bpool,
            tc.tile_pool(name="wpool", bufs=1) as wpool,
            tc.tile_pool(name="agpool", bufs=2) as agpool,
            tc.tile_pool(name="work", bufs=2) as work,
            tc.tile_pool(name="acc", bufs=1, space="PSUM") as accp,
            tc.tile_pool(name="tpp", bufs=1, space="PSUM") as tpp,
        ):
            xt8_sb = constp.tile([128, NT * 128], F8, name="xt8_sb")
            nc.sync.dma_start(xt8_sb, xt8[:])
            xt_sb = constp.tile([128, NT * 128], BF16, name="xt_sb")
            nc.sync.dma_start(xt_sb, xt[:])
            bias_sb = constp.tile([1, SH], BF16, name="bias_sb")
            nc.sync.dma_start(bias_sb, bias[:])
            ones = constp.tile([1, 128], BF16, name="ones")
            nc.vector.memset(ones, 1.0)

            xt_v = xt_sb.rearrange("p (i c) -> p i c", i=NT)
            xt8_v = xt8_sb.rearrange("p (j u c) -> p j u c", j=NT // 2, u=2)

            for _rep in range(reps):
                ut_sh = dramp.tile([SHK, 2, 128], F8, tag="ut_sh",
                                   name=f"ut_sh{_rep}")
                ut_ag = dramp.tile([KH, 2, 128], F8, tag="ut_ag",
                                   name=f"ut_ag{_rep}", addr_space="Shared")
                vt_sh = dramp.tile([SHK, 2, 128], F8, tag="vt_sh",
                                   name=f"vt_sh{_rep}")
                vt_ag = dramp.tile([KH, 2, 128], F8, tag="vt_ag",
                                   name=f"vt_ag{_rep}", addr_space="Shared")

                def load(pool, eng, src, shape, nm, dt=F8, sl=None):
                    t = pool.tile(shape, dt, tag=nm.rstrip("0123456789"),
                                  name=nm)
                    eng.dma_start(t, src[:] if sl is None else src[:, sl])
                    return t

                ar_lo = load(apool, nc.scalar, a_r, AH, f"arlo{_rep}",
                             sl=slice(0, AHW))
                ar_hi = load(apool, nc.scalar, a_r, AH, f"arhi{_rep}",
                             sl=slice(AHW, 2 * AHW))
                ai_lo = load(apool, nc.gpsimd, a_i, AH, f"ailo{_rep}",
                             sl=slice(0, AHW))
                ai_hi = load(apool, nc.gpsimd, a_i, AH, f"aihi{_rep}",
                             sl=slice(AHW, 2 * AHW))
                WH = NTH * SH
                wt_a = load(wpool, nc.sync, w_t, [128, WH], f"wta{_rep}",
                            BF16, sl=slice(0, WH))

                # ---- stage 1 (transposed): uT[k,t] = A[:,k].T @ xT -------
                # 6 PSUM tiles (3 row-tiles x re/im), fp8 DoubleRow over
                # d-tile pairs; each finished tile is cast straight to the
                # fp8 staging buffer.
                stg1 = work.tile([128, 3 * 256], F8, tag="stg",
                                 name=f"ustg{_rep}")

                def s1_chain(a_lo, a_hi, h):
                    for ti, (off, rows) in enumerate(TAUS):
                        pst = tpp.tile([128, 128], F32,
                                       tag=f"tp{(ti * 2 + h) % 4}",
                                       name=f"ups{_rep}_{ti}_{h}")
                        for j in range(NT // 2):
                            asrc = a_lo if j < NT // 4 else a_hi
                            jj = j if j < NT // 4 else j - NT // 4
                            lhsT = asrc[:, 2 * jj * SHK:
                                        (2 * jj + 2) * SHK].rearrange(
                                "p (u c) -> p u c", u=2)[:, :, off:off + rows]
                            nc.tensor.matmul(pst[:rows, :], lhsT, xt8_v[:, j],
                                             start=(j == 0),
                                             stop=(j == NT // 2 - 1),
                                             perf_mode=DR)
                        nc.vector.tensor_copy(
                            out=stg1[:rows, ti * 256 + h * 128:
                                     ti * 256 + h * 128 + 128],
                            in_=pst[:rows, :])

                s1_chain(ar_lo, ar_hi, 0)
                s1_chain(ai_lo, ai_hi, 1)

                def stage_dma(stg, sh_dram):
                    nc.scalar.dma_start(
                        out=sh_dram[0:256].rearrange("(j p) h c -> p j h c",
                                                     p=128),
                        in_=stg[:, 0:512].rearrange("p (j h c) -> p j h c",
                                                    j=2, h=2))
                    nc.scalar.dma_start(
                        out=sh_dram[256:272],
                        in_=stg[0:16, 512:768].rearrange("p (h c) -> p h c",
                                                         h=2))

                stage_dma(stg1, ut_sh)

                nc.gpsimd.collective_compute(
                    "AllGather", mybir.AluOpType.bypass,
                    ins=[ut_sh.opt()], outs=[ut_ag.opt()], replica_groups=RG)

                # B loads fill the AG_u window on the Pool queue; stage-2
                # weights queue behind the staging DMAs on Act
                bc_sb = load(bpool, nc.gpsimd, b_c, BS, f"bc{_rep}")
                bsn_sb = load(bpool, nc.gpsimd, b_sn, BS, f"bsn{_rep}")
                ppr_sb = load(pmpool, nc.scalar, ppr, PMS, f"ppr{_rep}")
                mpin_sb = load(pmpool, nc.scalar, mpin, PMS, f"mpin{_rep}")
                pmi_sb = load(pmpool, nc.scalar, pmi, PMS, f"pmi{_rep}")
                mmr_sb = load(pmpool, nc.scalar, mmr, PMS, f"mmr{_rep}")
                wt_b = load(wpool, nc.sync, w_t, [128, WH], f"wtb{_rep}",
                            BF16, sl=slice(WH, 2 * WH))

                # first half of the frozen-base matmul fills the AG_u stall
                ps_b = accp.tile([T, SH], F32, tag="s4", name=f"ps_b{_rep}")
                for i in range(NTH):
                    nc.tensor.matmul(ps_b, xt_v[:, i], wt_a[:, ts(i, SH)],
                                     start=(i == 0), stop=False)

                # bounce split 3 ways across SP/Act/Pool queues; stage 2
                # consumes pieces in order
                def bounce(ag, pfx):
                    views = []
                    engs = (nc.sync, nc.scalar, nc.gpsimd)
                    for pi, (t0, nt_p) in enumerate(PIECES):
                        t = agpool.tile([128, nt_p * 256], F8,
                                        tag=f"{pfx}{pi}",
                                        name=f"{pfx}{pi}_{_rep}")
                        engs[pi].dma_start(
                            out=t.rearrange("p (i c) -> p i c", i=nt_p),
                            in_=ag[t0 * 128: (t0 + nt_p) * 128].rearrange(
                                "(i p) h c -> p i (h c)", p=128))
                        views.append(t.rearrange("p (i h c) -> p i h c",
                                                 i=nt_p, h=2))
                    return tuple(views)

                ut_hv = bounce(ut_ag, "ut")

                # ---- stage 2 (transposed): vT[l,t] via stationary PM ------
                # vp = ur@P+r - ui@M+i ; vm = ur@P-i + ui@M-r, output [l, t].
                stg2 = work.tile([128, 3 * 256], F8, tag="stg",
                                 name=f"vstg{_rep}")
                s2mats = (((ppr_sb, 0), (mpin_sb, 1)),   # -> vp (h=0 slot)
                          ((pmi_sb, 0), (mmr_sb, 1)))    # -> vm (h=1 slot)
                for ti, (off, rows) in enumerate(TAUS):
                    for comp, mats in enumerate(s2mats):
                        pst = tpp.tile([128, 128], F32,
                                       tag=f"tp{(ti * 2 + comp) % 4}",
                                       name=f"vps{_rep}_{ti}_{comp}")
                        n_mm = 2 * len(GROUPS)
                        mm = 0
                        for mat, h in mats:
                            for li, half, loc in GROUPS:
                                dr = len(li) == 2
                                if dr:
                                    lhsT = mat[:, li[0] * SHK:
                                               (li[1] + 1) * SHK].rearrange(
                                        "p (u c) -> p u c",
                                        u=2)[:, :, off:off + rows]
                                    rhs = ut_hv[half][:, loc:loc + 2, h]
                                else:
                                    lhsT = mat[:, li[0] * SHK + off:
                                               li[0] * SHK + off + rows]
                                    rhs = ut_hv[half][:, loc, h]
                                nc.tensor.matmul(pst[:rows, :], lhsT, rhs,
                                                 start=(mm == 0),
                                                 stop=(mm == n_mm - 1),
                                                 perf_mode=DR if dr else None)
                                mm += 1
                        nc.vector.tensor_scalar_mul(
                            stg2[:rows, ti * 256 + comp * 128:
                                 ti * 256 + comp * 128 + 128],
                            pst[:rows, :], V_CAST)

                stage_dma(stg2, vt_sh)

                nc.gpsimd.collective_compute(
                    "AllGather", mybir.AluOpType.bypass,
                    ins=[vt_sh.opt()], outs=[vt_ag.opt()], replica_groups=RG)

                # second base half + bias fills the AG_v stall
                for i in range(NTH):
                    nc.tensor.matmul(ps_b, xt_v[:, NTH + i],
                                     wt_b[:, ts(i, SH)],
                                     start=False, stop=False)
                nc.tensor.matmul(ps_b, ones, bias_sb, start=False, stop=True)

                vt_hv = bounce(vt_ag, "vt")

                # ---- stage 3: y2 = vpT.T@Bc + vmT.T@Bsn, two column halves -
                for hf in range(2):
                    hoff = hf * (SH // 2)
                    ps_y2 = accp.tile([T, SH // 2], F32, tag=f"s3{hf}",
                                      name=f"ps_y2{_rep}_{hf}")
                    for gi, (li, half, loc) in enumerate(GROUPS):
                        dr = len(li) == 2
                        for mat, h in ((bc_sb, 0), (bsn_sb, 1)):
                            if dr:
                                rhs = mat[:, li[0] * SH:
                                          (li[1] + 1) * SH].rearrange(
                                    "p (u c) -> p u c",
                                    u=2)[:, :, hoff:hoff + SH // 2]
                                lhsT = vt_hv[half][:, loc:loc + 2, h]
                            else:
                                rhs = mat[:, li[0] * SH + hoff:
                                          li[0] * SH + hoff + SH // 2]
                                lhsT = vt_hv[half][:, loc, h]
                            nc.tensor.matmul(ps_y2, lhsT, rhs,
                                             start=(gi == 0),
                                             stop=(gi == len(GROUPS) - 1),
                                             perf_mode=DR if dr else None)

                    tmp = work.tile([T, SH // 2], F32, tag=f"tmpf{hf}",
                                    name=f"tmp{_rep}_{hf}")
                    nc.vector.tensor_scalar_mul(tmp, ps_y2, GAMMA)
                    y_sb = work.tile([T, SH // 2], BF16, tag=f"ysb{hf}",
                                     name=f"y_sb{_rep}_{hf}")
                    nc.vector.tensor_add(out=y_sb, in0=tmp,
                                         in1=ps_b[:, hoff:hoff + SH // 2])
                    nc.sync.dma_start(out=y_out[:, hoff:hoff + SH // 2],
                                      in_=y_sb)

    nc.compile()
    return nc


def _host_prep(x, W_base, b_base, c_re, c_im, mask_idx):
    xf = np.asarray(x, np.float32).reshape(T, D)
    xT = np.ascontiguousarray(xf.T)

    idx = np.arange(D, dtype=np.int64)
    tab_c = np.cos(2 * np.pi * np.arange(D) / D).astype(np.float32)
    tab_s = np.sin(2 * np.pi * np.arange(D) / D).astype(np.float32)

    # stage-1 matrices: 0.5 * cos/sin(2pi d k / D), cols 2049.. zero
    ph = (idx[:, None] * idx[None, :KH]) % D          # [4096, 2176]
    Ar = tab_c[ph] * A_SCALE
    Ai = tab_s[ph] * A_SCALE
    Ar[:, 2049:] = 0.0
    Ai[:, 2049:] = 0.0
    del ph

    # scatter c and fold both axes onto the half spectrum
    Fr = np.zeros(D * D, np.float32)
    Fi = np.zeros(D * D, np.float32)
    mi = np.asarray(mask_idx, np.int64)
    Fr[mi] = np.asarray(c_re, np.float32)
    Fi[mi] = np.asarray(c_im, np.float32)
    Fr = Fr.reshape(D, D)
    Fi = Fi.reshape(D, D)
    rev = (-np.arange(KH)) % D
    Pr = Fr[:KH] + Fr[rev]
    Pi = Fi[:KH] + Fi[rev]
    Mr = Fr[:KH] - Fr[rev]
    Mi = Fi[:KH] - Fi[rev]
    del Fr, Fi
    for X in (Pr, Pi, Mr, Mi):
        X[0] *= 0.5
        X[2048] *= 0.5
        X[2049:] = 0.0

    crev = (-np.arange(KH)) % D

    def lfold(X, sign):
        Y = X[:, :KH].copy()
        Y[:, :2049] += sign * X[:, crev[:2049]]
        Y[:, 0] *= 0.5
        Y[:, 2048] *= 0.5
        Y[:, 2049:] = 0.0
        return Y * PM_SCALE

    Ppr = lfold(Pr, +1.0)
    Mpin = -lfold(Mi, +1.0)
    Pmi = lfold(Pi, -1.0)
    Mmr = lfold(Mr, -1.0)
    del Pr, Pi, Mr, Mi

    # stage-3 matrices: cos/-sin(2pi l n / D), rows 2049.. zero
    ph3 = (np.arange(KH, dtype=np.int64)[:, None] * idx[None, :]) % D
    Bc = tab_c[ph3]
    Bsn = -tab_s[ph3]
    Bc[2049:] = 0.0
    Bsn[2049:] = 0.0
    del ph3

    Wb = np.asarray(W_base, np.float32)
    bb = np.asarray(b_base, np.float32)

    in_maps = []
    for m in range(NCORES):
        s = slice(m * SH, (m + 1) * SH)
        sk = slice(m * SHK, (m + 1) * SHK)
        in_maps.append({
            "xt": _tilemaj(xT, NP_BF16),
            "xt8": _tilemaj(xT, NP_F8),
            "a_r": _tilemaj(Ar[:, sk], NP_F8),
            "a_i": _tilemaj(Ai[:, sk], NP_F8),
            "ppr": _tilemaj(Ppr[:, sk], NP_F8),
            "mpin": _tilemaj(Mpin[:, sk], NP_F8),
            "pmi": _tilemaj(Pmi[:, sk], NP_F8),
            "mmr": _tilemaj(Mmr[:, sk], NP_F8),
            "b_c": _tilemaj(Bc[:, s], NP_F8),
            "b_sn": _tilemaj(Bsn[:, s], NP_F8),
            "w_t": _tilemaj(np.ascontiguousarray(Wb[s, :].T), NP_BF16),
            "bias": bb[s].reshape(1, SH).astype(NP_BF16),
        })
    return in_maps


def kernel(x, W_base, b_base, c_re, c_im, mask_idx, _trace=False):
    if "nc" not in _CACHE:
        _CACHE["nc"] = _build_program()
    nc = _CACHE["nc"]
    in_maps = _host_prep(x, W_base, b_base, c_re, c_im, mask_idx)
    res = run_bass_kernel_spmd(nc, in_maps, list(range(NCORES)), trace=_trace)
    _CACHE["last"] = res
    y = np.concatenate([res.results[m]["y"] for m in range(NCORES)], axis=1)
    return y.reshape(2, 64, D).astype(np.float32)
